# revision 17
# baseline (speedup 1.0000x reference)
"""Trainium2 Bass kernel for nn_DenoiserPairFeatures (v2).

Math: the [n,n,219] feature tensor is a concat of one-hots (seq-sep 127,
dist-bins 30+30) plus zero blocks, so feats @ W.T + b collapses to 3 table
gathers + bias, realized on the TensorEngine as 0/1-indicator matmuls
against host-precomputed compensated-cumulative bf16 tables.

v2 structure:
- Mask sparsity: only active rows (mask[i]=1) are computed, round-robin
  over 8 cores; only active columns are computed, packed densely into
  JTa j-tiles shared by all rows.  Host scatters results into the full
  [n,n,256] output (inactive pairs are exactly zero).
- LayerNorm mean is free: all table rows are mean-centered on host, so
  the gathered y is already mean-subtracted (linearity).  Only E[y^2]
  is reduced on device.
- The indicator matrices F are built by a single DVE compare
  (tensor_scalar is_gt) against per-partition thresholds, fed by small
  broadcast DMAs - no PE/Act work.
- Per 128-pair tile: PE matmul (K=126 band chain + K=62 bins chain) ->
  PSUM f32; Act copies PSUM->SBUF bf16; DVE square-reduces for E[y^2];
  Act sqrt + DVE reciprocal; DVE applies out = y * rsqrt(var+eps); DMA
  out in bf16 (host converts to f32).
"""

import os
import sys

sys.path.insert(0, "/opt/trn_rl_repo")

import numpy as np
import ml_dtypes

N = 1024
SEQ = 127          # seq-sep one-hot classes
NB = 30            # dist bins
C_OUT = 256
N_CORES = 8
LN_EPS = 1e-5
KB = 62            # B-matrix rows: 29 t + 29 sc + 2 sep-left + 2 bias
KA = 126           # A-matrix rows: sep chain

BF16 = ml_dtypes.bfloat16

_PROGRAM_CACHE = {}
LAST_PROFILE = None  # set when KERNEL_TRACE=1


def _bf16_f64(x):
    return np.asarray(x, np.float64).astype(BF16).astype(np.float64)


def _comp_chain01(T):
    """0/1-indicator compensated chain, single-level bf16.

    T: [M+1, C] float64 targets.  inc[k] bf16-realized so that partial sums
    sum(inc[0:s]) track T[s]-T[0] with non-accumulating ~bf16 error."""
    M = T.shape[0] - 1
    C = T.shape[1]
    P = np.zeros(C, np.float64)
    inc = np.empty((M, C), np.float64)
    for k in range(M):
        g = T[k + 1] - T[0] - P
        gq = _bf16_f64(g)
        inc[k] = gq
        P += gq
    return inc


def _dist_bins(coords):
    """Bin indices exactly as the reference computes them."""
    import jax.numpy as jnp

    edges = jnp.linspace(0.1, 3.0, NB - 1)
    x = jnp.asarray(np.asarray(coords, np.float32))
    diff = x[:, None, :] - x[None, :, :]
    d = jnp.sqrt(jnp.sum(jnp.square(diff), axis=-1) + 1e-10)
    return np.asarray(jnp.searchsorted(edges, d), dtype=np.int32)


def _build_tables(W, b):
    """Mean-centered chain tables.  ga [126,256] sep chain; gb [62,256]:
    t-chain 29, sc-chain 29, sep (T126-T0) hi+lo, base hi+lo."""
    W = np.asarray(W, np.float64)
    b = np.asarray(b, np.float64)
    Tsep = W[:, 0:SEQ].T.copy()
    Tt = W[:, SEQ:SEQ + NB].T.copy()
    Tsc = W[:, SEQ + NB:SEQ + 2 * NB].T.copy()
    Tsep -= Tsep.mean(axis=1, keepdims=True)
    Tt -= Tt.mean(axis=1, keepdims=True)
    Tsc -= Tsc.mean(axis=1, keepdims=True)
    bc = b - b.mean()

    ga = _comp_chain01(Tsep)                    # [126, 256]
    incT = _comp_chain01(Tt)                    # [29, 256]
    incS = _comp_chain01(Tsc)                   # [29, 256]
    diff = Tsep[SEQ - 1] - Tsep[0]
    d_hi = _bf16_f64(diff)
    d_lo = _bf16_f64(diff - d_hi)
    B0 = bc + Tsep[0] + Tt[0] + Tsc[0]
    b_hi = _bf16_f64(B0)
    b_lo = _bf16_f64(B0 - b_hi)
    gb = np.concatenate(
        [incT, incS, d_hi[None], d_lo[None], b_hi[None], b_lo[None]], axis=0)
    return ga.astype(BF16), gb.astype(BF16)


def _thresholds():
    tha = np.empty((KA, 1), np.float32)
    for p in range(KA):
        tha[p, 0] = p - 62.5            # F_A[p] = (i-j > p-62.5)
    thb = np.empty((KB, 1), np.float32)
    for k in range(29):
        thb[k, 0] = k + 0.5             # (tb > k+0.5)
        thb[29 + k, 0] = k + 0.5        # (sb > k+0.5)
    thb[58:60, 0] = 0.5                 # (-v > 0.5)  <=> p < 128*pb
    thb[60:62, 0] = 0.5                 # (1 > 0.5)   constant rows
    return tha, thb


def _build_program(R, JT):
    """Build + compile the SPMD program for R row-slots, JT packed j-tiles."""
    key = (R, JT)
    if key in _PROGRAM_CACHE:
        return _PROGRAM_CACHE[key]

    from concourse import bacc, mybir, tile

    P = JT * 128
    dt = mybir.dt
    nc = bacc.Bacc("TRN2", target_bir_lowering=False, debug=False,
                   num_devices=N_CORES)

    ga_d = nc.dram_tensor("ga", [KA, C_OUT], dt.bfloat16, kind="ExternalInput").ap()
    gb_d = nc.dram_tensor("gb", [KB, C_OUT], dt.bfloat16, kind="ExternalInput").ap()
    tha_d = nc.dram_tensor("tha", [KA, 1], dt.float32, kind="ExternalInput").ap()
    thb_d = nc.dram_tensor("thb", [KB, 1], dt.float32, kind="ExternalInput").ap()
    bcb_d = nc.dram_tensor("bcb", [R * KB, P], dt.bfloat16, kind="ExternalInput").ap()
    bca_d = nc.dram_tensor("bca", [R * KA, 256], dt.bfloat16, kind="ExternalInput").ap()
    # partition-major output: addr = ((r*128+p)*JT + q)*C_OUT + o
    out_d = nc.dram_tensor("out", [R, 128, JT * C_OUT], dt.bfloat16,
                           kind="ExternalOutput").ap()

    NBANK = (JT + 1) // 2

    with tile.TileContext(nc) as tc:
        with (
            tc.tile_pool(name="const", bufs=1) as cpool,
            tc.tile_pool(name="bc", bufs=3) as bcpool,
            tc.tile_pool(name="f", bufs=3) as fpool,
            tc.tile_pool(name="y", bufs=4, space="PSUM") as ypool,
            tc.tile_pool(name="st", bufs=4) as stpool,
            tc.tile_pool(name="ot", bufs=3) as opool,
        ):
            GA = cpool.tile([KA, C_OUT], dt.bfloat16)
            nc.sync.dma_start(out=GA[:], in_=ga_d[:])
            GB = cpool.tile([KB, C_OUT], dt.bfloat16)
            nc.sync.dma_start(out=GB[:], in_=gb_d[:])
            THA = cpool.tile([KA, 1], dt.float32)
            nc.sync.dma_start(out=THA[:], in_=tha_d[:])
            THB = cpool.tile([KB, 1], dt.float32)
            nc.sync.dma_start(out=THB[:], in_=thb_d[:])
            EPS = cpool.tile([128, 1], dt.float32)
            nc.vector.memset(EPS[:], LN_EPS)

            Sqrt = mybir.ActivationFunctionType.Sqrt
            Ident = mybir.ActivationFunctionType.Identity
            mult = mybir.AluOpType.mult
            is_gt = mybir.AluOpType.is_gt

            for r in range(R):
                # ---- stage per-row compare inputs (spread DMA queues) ----
                qa = nc.scalar if r % 2 == 0 else nc.gpsimd
                qb = nc.gpsimd if r % 2 == 0 else nc.scalar
                BCB = bcpool.tile([KB, P], dt.bfloat16, tag="bcb")
                qa.dma_start(out=BCB[:], in_=bcb_d[r * KB:(r + 1) * KB, :])
                BCA = bcpool.tile([KA, 256], dt.bfloat16, tag="bca")
                qb.dma_start(out=BCA[:], in_=bca_d[r * KA:(r + 1) * KA, :])

                # ---- indicator matrices via GpSimd compare ----
                FB = fpool.tile([KB, P], dt.bfloat16, tag="fb")
                nc.gpsimd.tensor_scalar(FB[:], BCB[:], THB[:, 0:1], None, op0=is_gt)
                FA = fpool.tile([KA, 256], dt.bfloat16, tag="fa")
                nc.gpsimd.tensor_scalar(FA[:], BCA[:], THA[:, 0:1], None, op0=is_gt)

                # ---- per-bank: matmuls; DVE bn_stats (batched 3D) ----
                ytiles = []
                MV = stpool.tile([128, JT, 2], dt.float32, tag="mv")
                for bank in range(NBANK):
                    ns = min(2, JT - 2 * bank)
                    Y = ypool.tile([128, 2, C_OUT], dt.float32, tag="y")
                    ytiles.append(Y)
                    for s in range(ns):
                        q = 2 * bank + s
                        if q < 2:
                            nc.tensor.matmul(Y[:, s, :],
                                             FA[:, q * 128:(q + 1) * 128],
                                             GA[:], start=True, stop=False)
                            nc.tensor.matmul(Y[:, s, :],
                                             FB[:, q * 128:(q + 1) * 128],
                                             GB[:], start=False, stop=True)
                        else:
                            nc.tensor.matmul(Y[:, s, :],
                                             FB[:, q * 128:(q + 1) * 128],
                                             GB[:], start=True, stop=True)
                    ST = stpool.tile([128, 2, 6], dt.float32, tag="st")
                    for s in range(ns):
                        q = 2 * bank + s
                        nc.vector.bn_stats(ST[:, s, :], Y[:, s, :])
                        nc.vector.bn_aggr(MV[:, q, :], ST[:, s, :])

                # ---- rstd + apply (Act, per-partition scale) ----
                SD1 = stpool.tile([128, JT], dt.float32, tag="sd1")
                nc.scalar.activation(SD1[:], MV[:, :, 1], Sqrt, bias=EPS[:, 0:1])
                SD = stpool.tile([128, JT], dt.float32, tag="sd")
                nc.vector.reciprocal(SD[:], SD1[:])
                OT = opool.tile([128, JT * C_OUT], dt.bfloat16, tag="ot")
                for q in range(JT):
                    nc.scalar.activation(
                        OT[:, q * C_OUT:(q + 1) * C_OUT],
                        ytiles[q // 2][:, q % 2, :],
                        Ident, bias=0.0, scale=SD[:, q:q + 1])

                # ---- output DMA: partition-major, contiguous per partition ----
                nc.sync.dma_start(out=out_d[r], in_=OT[:])

    nc.compile()
    _PROGRAM_CACHE[key] = nc
    return nc


def _host_data(mask, x_t, x_sc, W, b):
    """Active-row/col packing, per-core compare inputs, tables."""
    mask = np.asarray(mask)
    act = mask != 0
    A = np.flatnonzero(act)
    nA = int(len(A))
    if nA == 0:
        return None
    JT = max(2, (nA + 127) // 128)
    P = JT * 128
    Ap = np.concatenate([A, np.full(P - nA, A[-1], dtype=A.dtype)])
    Rc = (nA + N_CORES - 1) // N_CORES

    ga, gb = _build_tables(W, b)
    tha, thb = _thresholds()
    tb = _dist_bins(x_t)
    sb = _dist_bins(x_sc)

    pos = np.arange(P)
    pos_t = pos // 128
    pos_p = pos % 128

    cores = []
    meta = []
    for c in range(N_CORES):
        rows_c = A[c::N_CORES]
        nr = len(rows_c)
        rows = np.full(Rc, rows_c[-1] if nr else A[0], dtype=np.int64)
        rows[:nr] = rows_c

        lo = np.searchsorted(A, rows - 62, side="left")
        pb = np.clip(lo // 128, 0, JT - 2)

        # processed position -> packed index (rotation by pb tiles)
        ptrue = ((pb[:, None] + pos_t[None, :]) % JT) * 128 + pos_p[None, :]
        jtrue = Ap[ptrue]                          # [Rc, P] true col ids

        tbv = tb[rows[:, None], jtrue]             # [Rc, P]
        sbv = sb[rows[:, None], jtrue]
        vv = ptrue - 128 * pb[:, None]             # packed idx - window start
        bcb = np.empty((Rc, KB, P), np.float32)
        bcb[:, 0:29, :] = tbv[:, None, :]
        bcb[:, 29:58, :] = sbv[:, None, :]
        bcb[:, 58:60, :] = -vv[:, None, :]
        bcb[:, 60:62, :] = 1.0

        jwin = jtrue[:, 0:256]                     # window cols (natural order)
        u = (rows[:, None] - jwin).astype(np.float32)  # i - j
        bca = np.broadcast_to(u[:, None, :], (Rc, KA, 256))

        cores.append({
            "ga": np.ascontiguousarray(ga),
            "gb": np.ascontiguousarray(gb),
            "tha": tha,
            "thb": thb,
            "bcb": np.ascontiguousarray(bcb.reshape(Rc * KB, P)).astype(BF16),
            "bca": np.ascontiguousarray(bca.reshape(Rc * KA, 256)).astype(BF16),
        })
        meta.append((rows_c, pb[:nr] if nr else pb[:0]))
    return cores, meta, A, nA, Rc, JT


def kernel(mask, x_t, x_sc, W, b, gamma, beta):
    global LAST_PROFILE
    from concourse.bass_utils import run_bass_kernel_spmd

    mask = np.asarray(mask)
    out = np.zeros((N, N, C_OUT), np.float32)
    host = _host_data(mask, x_t, x_sc, W, b)
    if host is not None:
        cores, meta, A, nA, Rc, JT = host
        P = JT * 128
        nc = _build_program(Rc, JT)

        trace = bool(int(os.environ.get("KERNEL_TRACE", "0")))
        tdir = os.environ.get("KERNEL_TRACE_DIR") or None
        if tdir:
            os.makedirs(tdir, exist_ok=True)
        res = run_bass_kernel_spmd(nc, cores, list(range(N_CORES)), trace=trace,
                                   tmpdir=tdir)
        LAST_PROFILE = res

        for c in range(N_CORES):
            oc = res.results[c]["out"]             # [Rc, 128, JT*256] bf16
            rows_c, pbs = meta[c]
            for r, (i, pbr) in enumerate(zip(rows_c, pbs)):
                blk = oc[r].reshape(128, JT, C_OUT).transpose(1, 0, 2)
                if pbr:
                    blk = np.roll(blk, pbr, axis=0)
                out[i, A] = blk.reshape(P, C_OUT)[:nA].astype(np.float32)

    gamma = np.asarray(gamma, np.float32)
    beta = np.asarray(beta, np.float32)
    if not (np.all(gamma == 1.0) and np.all(beta == 0.0)):
        pm = (mask.astype(np.float32)[:, None] * mask.astype(np.float32)[None, :])
        out = out * gamma[None, None, :] + pm[:, :, None] * beta[None, None, :]
    return out


# revision 18
# speedup vs baseline: 3.3567x; 3.3567x over previous
"""Trainium2 Bass kernel for nn_DenoiserPairFeatures (v2).

Math: the [n,n,219] feature tensor is a concat of one-hots (seq-sep 127,
dist-bins 30+30) plus zero blocks, so feats @ W.T + b collapses to 3 table
gathers + bias, realized on the TensorEngine as 0/1-indicator matmuls
against host-precomputed compensated-cumulative bf16 tables.

v2 structure:
- Mask sparsity: only active rows (mask[i]=1) are computed, round-robin
  over 8 cores; only active columns are computed, packed densely into
  JTa j-tiles shared by all rows.  Host scatters results into the full
  [n,n,256] output (inactive pairs are exactly zero).
- LayerNorm mean is free: all table rows are mean-centered on host, so
  the gathered y is already mean-subtracted (linearity).  Only E[y^2]
  is reduced on device.
- The indicator matrices F are built by a single DVE compare
  (tensor_scalar is_gt) against per-partition thresholds, fed by small
  broadcast DMAs - no PE/Act work.
- Per 128-pair tile: PE matmul (K=126 band chain + K=62 bins chain) ->
  PSUM f32; Act copies PSUM->SBUF bf16; DVE square-reduces for E[y^2];
  Act sqrt + DVE reciprocal; DVE applies out = y * rsqrt(var+eps); DMA
  out in bf16 (host converts to f32).
"""

import os
import sys

sys.path.insert(0, "/opt/trn_rl_repo")

import numpy as np
import ml_dtypes

N = 1024
SEQ = 127          # seq-sep one-hot classes
NB = 30            # dist bins
C_OUT = 256
N_CORES = 8
LN_EPS = 1e-5
KB = 62            # B-matrix rows: 29 t + 29 sc + 2 sep-left + 2 bias
KA = 126           # A-matrix rows: sep chain

BF16 = ml_dtypes.bfloat16

_PROGRAM_CACHE = {}
LAST_PROFILE = None  # set when KERNEL_TRACE=1


def _bf16_f64(x):
    return np.asarray(x, np.float64).astype(BF16).astype(np.float64)


def _comp_chain01(T):
    """0/1-indicator compensated chain, single-level bf16.

    T: [M+1, C] float64 targets.  inc[k] bf16-realized so that partial sums
    sum(inc[0:s]) track T[s]-T[0] with non-accumulating ~bf16 error."""
    M = T.shape[0] - 1
    C = T.shape[1]
    P = np.zeros(C, np.float64)
    inc = np.empty((M, C), np.float64)
    for k in range(M):
        g = T[k + 1] - T[0] - P
        gq = _bf16_f64(g)
        inc[k] = gq
        P += gq
    return inc


def _dist_bins(coords):
    """Bin indices exactly as the reference computes them."""
    import jax.numpy as jnp

    edges = jnp.linspace(0.1, 3.0, NB - 1)
    x = jnp.asarray(np.asarray(coords, np.float32))
    diff = x[:, None, :] - x[None, :, :]
    d = jnp.sqrt(jnp.sum(jnp.square(diff), axis=-1) + 1e-10)
    return np.asarray(jnp.searchsorted(edges, d), dtype=np.int32)


def _build_tables(W, b):
    """Mean-centered chain tables.  ga [126,256] sep chain; gb [62,256]:
    t-chain 29, sc-chain 29, sep (T126-T0) hi+lo, base hi+lo."""
    W = np.asarray(W, np.float64)
    b = np.asarray(b, np.float64)
    Tsep = W[:, 0:SEQ].T.copy()
    Tt = W[:, SEQ:SEQ + NB].T.copy()
    Tsc = W[:, SEQ + NB:SEQ + 2 * NB].T.copy()
    Tsep -= Tsep.mean(axis=1, keepdims=True)
    Tt -= Tt.mean(axis=1, keepdims=True)
    Tsc -= Tsc.mean(axis=1, keepdims=True)
    bc = b - b.mean()

    ga = _comp_chain01(Tsep)                    # [126, 256]
    incT = _comp_chain01(Tt)                    # [29, 256]
    incS = _comp_chain01(Tsc)                   # [29, 256]
    diff = Tsep[SEQ - 1] - Tsep[0]
    d_hi = _bf16_f64(diff)
    d_lo = _bf16_f64(diff - d_hi)
    B0 = bc + Tsep[0] + Tt[0] + Tsc[0]
    b_hi = _bf16_f64(B0)
    b_lo = _bf16_f64(B0 - b_hi)
    gb = np.concatenate(
        [incT, incS, d_hi[None], d_lo[None], b_hi[None], b_lo[None]], axis=0)
    return ga.astype(BF16), gb.astype(BF16)


def _thresholds():
    tha = np.empty((KA, 1), np.float32)
    for p in range(KA):
        tha[p, 0] = p - 62.5            # F_A[p] = (i-j > p-62.5)
    thb = np.empty((KB, 1), np.float32)
    for k in range(29):
        thb[k, 0] = k + 0.5             # (tb > k+0.5)
        thb[29 + k, 0] = k + 0.5        # (sb > k+0.5)
    thb[58:60, 0] = 0.5                 # (-v > 0.5)  <=> p < 128*pb
    thb[60:62, 0] = 0.5                 # (1 > 0.5)   constant rows
    return tha, thb


def _build_program(R, JT):
    """Build + compile the SPMD program for R row-slots, JT packed j-tiles."""
    key = (R, JT)
    if key in _PROGRAM_CACHE:
        return _PROGRAM_CACHE[key]

    from concourse import bacc, mybir, tile

    P = JT * 128
    dt = mybir.dt
    nc = bacc.Bacc("TRN2", target_bir_lowering=False, debug=False,
                   num_devices=N_CORES)

    ga_d = nc.dram_tensor("ga", [KA, C_OUT], dt.bfloat16, kind="ExternalInput").ap()
    gb_d = nc.dram_tensor("gb", [KB, C_OUT], dt.bfloat16, kind="ExternalInput").ap()
    tha_d = nc.dram_tensor("tha", [KA, 1], dt.float32, kind="ExternalInput").ap()
    thb_d = nc.dram_tensor("thb", [KB, 1], dt.float32, kind="ExternalInput").ap()
    bcb_d = nc.dram_tensor("bcb", [R * KB, P], dt.bfloat16, kind="ExternalInput").ap()
    bca_d = nc.dram_tensor("bca", [R * KA, 256], dt.bfloat16, kind="ExternalInput").ap()
    # partition-major output: addr = ((r*128+p)*JT + q)*C_OUT + o
    out_d = nc.dram_tensor("out", [R, 128, JT * C_OUT], dt.bfloat16,
                           kind="ExternalOutput").ap()

    NBANK = (JT + 1) // 2

    with tile.TileContext(nc) as tc:
        with (
            tc.tile_pool(name="const", bufs=1) as cpool,
            tc.tile_pool(name="bc", bufs=3) as bcpool,
            tc.tile_pool(name="f", bufs=3) as fpool,
            tc.tile_pool(name="y", bufs=4, space="PSUM") as ypool,
            tc.tile_pool(name="st", bufs=4) as stpool,
            tc.tile_pool(name="ot", bufs=3) as opool,
        ):
            GA = cpool.tile([KA, C_OUT], dt.bfloat16)
            nc.sync.dma_start(out=GA[:], in_=ga_d[:])
            GB = cpool.tile([KB, C_OUT], dt.bfloat16)
            nc.sync.dma_start(out=GB[:], in_=gb_d[:])
            THA = cpool.tile([KA, 1], dt.float32)
            nc.sync.dma_start(out=THA[:], in_=tha_d[:])
            THB = cpool.tile([KB, 1], dt.float32)
            nc.sync.dma_start(out=THB[:], in_=thb_d[:])
            EPS = cpool.tile([128, 1], dt.float32)
            nc.vector.memset(EPS[:], LN_EPS)

            Sqrt = mybir.ActivationFunctionType.Sqrt
            Ident = mybir.ActivationFunctionType.Identity
            mult = mybir.AluOpType.mult
            is_gt = mybir.AluOpType.is_gt

            for r in range(R):
                # ---- stage per-row compare inputs (spread DMA queues) ----
                qa = nc.scalar if r % 2 == 0 else nc.gpsimd
                qb = nc.gpsimd if r % 2 == 0 else nc.scalar
                BCB = bcpool.tile([KB, P], dt.bfloat16, tag="bcb")
                qa.dma_start(out=BCB[:], in_=bcb_d[r * KB:(r + 1) * KB, :])
                BCA = bcpool.tile([KA, 256], dt.bfloat16, tag="bca")
                qb.dma_start(out=BCA[:], in_=bca_d[r * KA:(r + 1) * KA, :])

                # ---- indicator matrices via DVE compare ----
                FB = fpool.tile([KB, P], dt.bfloat16, tag="fb")
                nc.vector.tensor_scalar(FB[:], BCB[:], THB[:, 0:1], None, op0=is_gt)
                FA = fpool.tile([KA, 256], dt.bfloat16, tag="fa")
                nc.vector.tensor_scalar(FA[:], BCA[:], THA[:, 0:1], None, op0=is_gt)

                # ---- per-bank: matmuls; DVE bn_stats (batched 3D) ----
                ytiles = []
                MV = stpool.tile([128, JT, 2], dt.float32, tag="mv")
                for bank in range(NBANK):
                    ns = min(2, JT - 2 * bank)
                    Y = ypool.tile([128, 2, C_OUT], dt.float32, tag="y")
                    ytiles.append(Y)
                    for s in range(ns):
                        q = 2 * bank + s
                        if q < 2:
                            nc.tensor.matmul(Y[:, s, :],
                                             FA[:, q * 128:(q + 1) * 128],
                                             GA[:], start=True, stop=False)
                            nc.tensor.matmul(Y[:, s, :],
                                             FB[:, q * 128:(q + 1) * 128],
                                             GB[:], start=False, stop=True)
                        else:
                            nc.tensor.matmul(Y[:, s, :],
                                             FB[:, q * 128:(q + 1) * 128],
                                             GB[:], start=True, stop=True)
                    ST = stpool.tile([128, 2, 6], dt.float32, tag="st")
                    for s in range(ns):
                        q = 2 * bank + s
                        nc.vector.bn_stats(ST[:, s, :], Y[:, s, :])
                        nc.vector.bn_aggr(MV[:, q, :], ST[:, s, :])

                # ---- rstd + apply (Act, per-partition scale) ----
                SD1 = stpool.tile([128, JT], dt.float32, tag="sd1")
                nc.scalar.activation(SD1[:], MV[:, :, 1], Sqrt, bias=EPS[:, 0:1])
                SD = stpool.tile([128, JT], dt.float32, tag="sd")
                nc.vector.reciprocal(SD[:], SD1[:])
                OT = opool.tile([128, JT * C_OUT], dt.bfloat16, tag="ot")
                for q in range(JT):
                    nc.scalar.activation(
                        OT[:, q * C_OUT:(q + 1) * C_OUT],
                        ytiles[q // 2][:, q % 2, :],
                        Ident, bias=0.0, scale=SD[:, q:q + 1])

                # ---- output DMA: partition-major, contiguous per partition ----
                nc.sync.dma_start(out=out_d[r], in_=OT[:])

    nc.compile()
    _PROGRAM_CACHE[key] = nc
    return nc


def _host_data(mask, x_t, x_sc, W, b):
    """Active-row/col packing, per-core compare inputs, tables."""
    mask = np.asarray(mask)
    act = mask != 0
    A = np.flatnonzero(act)
    nA = int(len(A))
    if nA == 0:
        return None
    JT = max(2, (nA + 127) // 128)
    P = JT * 128
    Ap = np.concatenate([A, np.full(P - nA, A[-1], dtype=A.dtype)])
    Rc = (nA + N_CORES - 1) // N_CORES

    ga, gb = _build_tables(W, b)
    tha, thb = _thresholds()
    tb = _dist_bins(x_t)
    sb = _dist_bins(x_sc)

    pos = np.arange(P)
    pos_t = pos // 128
    pos_p = pos % 128

    cores = []
    meta = []
    for c in range(N_CORES):
        rows_c = A[c::N_CORES]
        nr = len(rows_c)
        rows = np.full(Rc, rows_c[-1] if nr else A[0], dtype=np.int64)
        rows[:nr] = rows_c

        lo = np.searchsorted(A, rows - 62, side="left")
        pb = np.clip(lo // 128, 0, JT - 2)

        # processed position -> packed index (rotation by pb tiles)
        ptrue = ((pb[:, None] + pos_t[None, :]) % JT) * 128 + pos_p[None, :]
        jtrue = Ap[ptrue]                          # [Rc, P] true col ids

        tbv = tb[rows[:, None], jtrue]             # [Rc, P]
        sbv = sb[rows[:, None], jtrue]
        vv = ptrue - 128 * pb[:, None]             # packed idx - window start
        bcb = np.empty((Rc, KB, P), np.float32)
        bcb[:, 0:29, :] = tbv[:, None, :]
        bcb[:, 29:58, :] = sbv[:, None, :]
        bcb[:, 58:60, :] = -vv[:, None, :]
        bcb[:, 60:62, :] = 1.0

        jwin = jtrue[:, 0:256]                     # window cols (natural order)
        u = (rows[:, None] - jwin).astype(np.float32)  # i - j
        bca = np.broadcast_to(u[:, None, :], (Rc, KA, 256))

        cores.append({
            "ga": np.ascontiguousarray(ga),
            "gb": np.ascontiguousarray(gb),
            "tha": tha,
            "thb": thb,
            "bcb": np.ascontiguousarray(bcb.reshape(Rc * KB, P)).astype(BF16),
            "bca": np.ascontiguousarray(bca.reshape(Rc * KA, 256)).astype(BF16),
        })
        meta.append((rows_c, pb[:nr] if nr else pb[:0]))
    return cores, meta, A, nA, Rc, JT


def kernel(mask, x_t, x_sc, W, b, gamma, beta):
    global LAST_PROFILE
    from concourse.bass_utils import run_bass_kernel_spmd

    mask = np.asarray(mask)
    out = np.zeros((N, N, C_OUT), np.float32)
    host = _host_data(mask, x_t, x_sc, W, b)
    if host is not None:
        cores, meta, A, nA, Rc, JT = host
        P = JT * 128
        nc = _build_program(Rc, JT)

        trace = bool(int(os.environ.get("KERNEL_TRACE", "0")))
        tdir = os.environ.get("KERNEL_TRACE_DIR") or None
        if tdir:
            os.makedirs(tdir, exist_ok=True)
        res = run_bass_kernel_spmd(nc, cores, list(range(N_CORES)), trace=trace,
                                   tmpdir=tdir)
        LAST_PROFILE = res

        for c in range(N_CORES):
            oc = res.results[c]["out"]             # [Rc, 128, JT*256] bf16
            rows_c, pbs = meta[c]
            for r, (i, pbr) in enumerate(zip(rows_c, pbs)):
                blk = oc[r].reshape(128, JT, C_OUT).transpose(1, 0, 2)
                if pbr:
                    blk = np.roll(blk, pbr, axis=0)
                out[i, A] = blk.reshape(P, C_OUT)[:nA].astype(np.float32)

    gamma = np.asarray(gamma, np.float32)
    beta = np.asarray(beta, np.float32)
    if not (np.all(gamma == 1.0) and np.all(beta == 0.0)):
        pm = (mask.astype(np.float32)[:, None] * mask.astype(np.float32)[None, :])
        out = out * gamma[None, None, :] + pm[:, :, None] * beta[None, None, :]
    return out


# revision 24
# speedup vs baseline: 3.4522x; 1.0285x over previous
"""Trainium2 Bass kernel for nn_DenoiserPairFeatures (v2).

Math: the [n,n,219] feature tensor is a concat of one-hots (seq-sep 127,
dist-bins 30+30) plus zero blocks, so feats @ W.T + b collapses to 3 table
gathers + bias, realized on the TensorEngine as 0/1-indicator matmuls
against host-precomputed compensated-cumulative bf16 tables.

v2 structure:
- Mask sparsity: only active rows (mask[i]=1) are computed, round-robin
  over 8 cores; only active columns are computed, packed densely into
  JTa j-tiles shared by all rows.  Host scatters results into the full
  [n,n,256] output (inactive pairs are exactly zero).
- LayerNorm mean is free: all table rows are mean-centered on host, so
  the gathered y is already mean-subtracted (linearity).  Only E[y^2]
  is reduced on device.
- The indicator matrices F are built by a single DVE compare
  (tensor_scalar is_gt) against per-partition thresholds, fed by small
  broadcast DMAs - no PE/Act work.
- Per 128-pair tile: PE matmul (K=126 band chain + K=62 bins chain) ->
  PSUM f32; Act copies PSUM->SBUF bf16; DVE square-reduces for E[y^2];
  Act sqrt + DVE reciprocal; DVE applies out = y * rsqrt(var+eps); DMA
  out in bf16 (host converts to f32).
"""

import os
import sys

sys.path.insert(0, "/opt/trn_rl_repo")

import numpy as np
import ml_dtypes

N = 1024
SEQ = 127          # seq-sep one-hot classes
NB = 30            # dist bins
C_OUT = 256
N_CORES = 8
LN_EPS = 1e-5
KB = 62            # B-matrix rows: 29 t + 29 sc + 2 sep-left + 2 bias
KA = 126           # A-matrix rows: sep chain

BF16 = ml_dtypes.bfloat16

_PROGRAM_CACHE = {}
LAST_PROFILE = None  # set when KERNEL_TRACE=1


def _bf16_f64(x):
    return np.asarray(x, np.float64).astype(BF16).astype(np.float64)


def _comp_chain01(T):
    """0/1-indicator compensated chain, single-level bf16.

    T: [M+1, C] float64 targets.  inc[k] bf16-realized so that partial sums
    sum(inc[0:s]) track T[s]-T[0] with non-accumulating ~bf16 error."""
    M = T.shape[0] - 1
    C = T.shape[1]
    P = np.zeros(C, np.float64)
    inc = np.empty((M, C), np.float64)
    for k in range(M):
        g = T[k + 1] - T[0] - P
        gq = _bf16_f64(g)
        inc[k] = gq
        P += gq
    return inc


def _dist_bins(coords):
    """Bin indices exactly as the reference computes them."""
    import jax.numpy as jnp

    edges = jnp.linspace(0.1, 3.0, NB - 1)
    x = jnp.asarray(np.asarray(coords, np.float32))
    diff = x[:, None, :] - x[None, :, :]
    d = jnp.sqrt(jnp.sum(jnp.square(diff), axis=-1) + 1e-10)
    return np.asarray(jnp.searchsorted(edges, d), dtype=np.int32)


def _build_tables(W, b):
    """Mean-centered chain tables.  ga [126,256] sep chain; gb [62,256]:
    t-chain 29, sc-chain 29, sep (T126-T0) hi+lo, base hi+lo."""
    W = np.asarray(W, np.float64)
    b = np.asarray(b, np.float64)
    Tsep = W[:, 0:SEQ].T.copy()
    Tt = W[:, SEQ:SEQ + NB].T.copy()
    Tsc = W[:, SEQ + NB:SEQ + 2 * NB].T.copy()
    Tsep -= Tsep.mean(axis=1, keepdims=True)
    Tt -= Tt.mean(axis=1, keepdims=True)
    Tsc -= Tsc.mean(axis=1, keepdims=True)
    bc = b - b.mean()

    ga = _comp_chain01(Tsep)                    # [126, 256]
    incT = _comp_chain01(Tt)                    # [29, 256]
    incS = _comp_chain01(Tsc)                   # [29, 256]
    diff = Tsep[SEQ - 1] - Tsep[0]
    d_hi = _bf16_f64(diff)
    d_lo = _bf16_f64(diff - d_hi)
    B0 = bc + Tsep[0] + Tt[0] + Tsc[0]
    b_hi = _bf16_f64(B0)
    b_lo = _bf16_f64(B0 - b_hi)
    gb = np.concatenate(
        [incT, incS, d_hi[None], d_lo[None], b_hi[None], b_lo[None]], axis=0)
    return ga.astype(BF16), gb.astype(BF16)


def _thresholds():
    tha = np.empty((KA, 1), np.float32)
    for p in range(KA):
        tha[p, 0] = p - 62.5            # F_A[p] = (i-j > p-62.5)
    thb = np.empty((KB, 1), np.float32)
    for k in range(29):
        thb[k, 0] = k + 0.5             # (tb > k+0.5)
        thb[29 + k, 0] = k + 0.5        # (sb > k+0.5)
    thb[58:60, 0] = 0.5                 # (-v > 0.5)  <=> p < 128*pb
    thb[60:62, 0] = 0.5                 # (1 > 0.5)   constant rows
    return tha, thb


def _build_program(R, JT):
    """Build + compile the SPMD program for R row-slots, JT packed j-tiles."""
    key = (R, JT)
    if key in _PROGRAM_CACHE:
        return _PROGRAM_CACHE[key]

    from concourse import bacc, mybir, tile

    P = JT * 128
    dt = mybir.dt
    nc = bacc.Bacc("TRN2", target_bir_lowering=False, debug=False,
                   num_devices=N_CORES)

    ga_d = nc.dram_tensor("ga", [KA, C_OUT], dt.bfloat16, kind="ExternalInput").ap()
    gb_d = nc.dram_tensor("gb", [KB, C_OUT], dt.bfloat16, kind="ExternalInput").ap()
    tha_d = nc.dram_tensor("tha", [KA, 1], dt.float32, kind="ExternalInput").ap()
    thb_d = nc.dram_tensor("thb", [KB, 1], dt.float32, kind="ExternalInput").ap()
    R2 = R // 2
    bcb_d = nc.dram_tensor("bcb", [R2 * KB, 2 * P], dt.bfloat16,
                           kind="ExternalInput").ap()
    bca_d = nc.dram_tensor("bca", [R2 * KA, 512], dt.bfloat16,
                           kind="ExternalInput").ap()
    # partition-major output: addr = ((r*128+p)*JT + q)*C_OUT + o
    out_d = nc.dram_tensor("out", [R, 128, JT * C_OUT], dt.bfloat16,
                           kind="ExternalOutput").ap()

    NBANK = (JT + 1) // 2

    with tile.TileContext(nc) as tc:
        with (
            tc.tile_pool(name="const", bufs=1) as cpool,
            tc.tile_pool(name="bc", bufs=3) as bcpool,
            tc.tile_pool(name="f", bufs=3) as fpool,
            tc.tile_pool(name="y", bufs=6, space="PSUM") as ypool,
            tc.tile_pool(name="st", bufs=4) as stpool,
            tc.tile_pool(name="ot", bufs=3) as opool,
        ):
            GA = cpool.tile([KA, C_OUT], dt.bfloat16)
            nc.sync.dma_start(out=GA[:], in_=ga_d[:])
            GB = cpool.tile([KB, C_OUT], dt.bfloat16)
            nc.sync.dma_start(out=GB[:], in_=gb_d[:])
            THA = cpool.tile([KA, 1], dt.float32)
            nc.sync.dma_start(out=THA[:], in_=tha_d[:])
            THB = cpool.tile([KB, 1], dt.float32)
            nc.sync.dma_start(out=THB[:], in_=thb_d[:])
            Sqrt = mybir.ActivationFunctionType.Sqrt
            Ident = mybir.ActivationFunctionType.Identity
            mult = mybir.AluOpType.mult
            is_gt = mybir.AluOpType.is_gt

            sub = mybir.AluOpType.subtract
            add = mybir.AluOpType.add

            for rp in range(R2):
                # ---- stage pair compare inputs (GpSimd DMA queue) ----
                BCB = bcpool.tile([KB, 2 * P], dt.bfloat16, tag="bcb")
                nc.gpsimd.dma_start(out=BCB[:],
                                    in_=bcb_d[rp * KB:(rp + 1) * KB, :])
                BCA = bcpool.tile([KA, 512], dt.bfloat16, tag="bca")
                nc.gpsimd.dma_start(out=BCA[:],
                                    in_=bca_d[rp * KA:(rp + 1) * KA, :])

                # ---- indicator matrices via DVE compare (pair-batched) ----
                FB = fpool.tile([KB, 2 * P], dt.bfloat16, tag="fb")
                nc.vector.tensor_scalar(FB[:], BCB[:], THB[:, 0:1], None, op0=is_gt)
                FA = fpool.tile([KA, 512], dt.bfloat16, tag="fa")
                nc.vector.tensor_scalar(FA[:], BCA[:], THA[:, 0:1], None, op0=is_gt)

                # ---- per row of pair: matmuls + bn_stats ----
                ST = stpool.tile([128, 2 * JT, 6], dt.float32, tag="st")
                ytiles = []
                for rloc in range(2):
                    for bank in range(NBANK):
                        ns = min(2, JT - 2 * bank)
                        Y = ypool.tile([128, 2, C_OUT], dt.float32, tag="y")
                        ytiles.append(Y)
                        for s in range(ns):
                            q = 2 * bank + s
                            fboff = rloc * P + q * 128
                            if q < 2:
                                faoff = rloc * 256 + q * 128
                                nc.tensor.matmul(Y[:, s, :],
                                                 FA[:, faoff:faoff + 128],
                                                 GA[:], start=True, stop=False)
                                nc.tensor.matmul(Y[:, s, :],
                                                 FB[:, fboff:fboff + 128],
                                                 GB[:], start=False, stop=True)
                            else:
                                nc.tensor.matmul(Y[:, s, :],
                                                 FB[:, fboff:fboff + 128],
                                                 GB[:], start=True, stop=True)
                            nc.vector.bn_stats(ST[:, rloc * JT + q, :], Y[:, s, :])

                # ---- pair-batched rstd: var = (cve+cvo)/256 + ((me-mo)/2)^2 ----
                DD = stpool.tile([128, 2 * JT], dt.float32, tag="dd")
                nc.vector.tensor_tensor(DD[:], ST[:, :, 1], ST[:, :, 4], op=sub)
                SS = stpool.tile([128, 2 * JT], dt.float32, tag="ss")
                nc.vector.tensor_tensor(SS[:], ST[:, :, 2], ST[:, :, 5], op=add)
                D24 = stpool.tile([128, 2 * JT], dt.float32, tag="d24")
                nc.vector.scalar_tensor_tensor(D24[:], DD[:], 0.25, DD[:],
                                               op0=mult, op1=mult)
                VV = stpool.tile([128, 2 * JT], dt.float32, tag="vv")
                nc.vector.scalar_tensor_tensor(VV[:], SS[:], 1.0 / C_OUT, D24[:],
                                               op0=mult, op1=add)
                RV = stpool.tile([128, 2 * JT], dt.float32, tag="rv")
                nc.vector.reciprocal(RV[:], VV[:])
                SD = stpool.tile([128, 2 * JT], dt.float32, tag="sd")
                nc.scalar.activation(SD[:], RV[:], Sqrt)

                # ---- applies (Act) + per-row output DMA ----
                for rloc in range(2):
                    OT = opool.tile([128, JT * C_OUT], dt.bfloat16, tag="ot")
                    for q in range(JT):
                        idx = rloc * JT + q
                        nc.scalar.activation(
                            OT[:, q * C_OUT:(q + 1) * C_OUT],
                            ytiles[rloc * NBANK + q // 2][:, q % 2, :],
                            Ident, bias=0.0, scale=SD[:, idx:idx + 1])
                    nc.sync.dma_start(out=out_d[2 * rp + rloc], in_=OT[:])

    nc.compile()
    _PROGRAM_CACHE[key] = nc
    return nc


def _host_data(mask, x_t, x_sc, W, b):
    """Active-row/col packing, per-core compare inputs, tables."""
    mask = np.asarray(mask)
    act = mask != 0
    A = np.flatnonzero(act)
    nA = int(len(A))
    if nA == 0:
        return None
    JT = max(2, (nA + 127) // 128)
    P = JT * 128
    Ap = np.concatenate([A, np.full(P - nA, A[-1], dtype=A.dtype)])
    Rc = (nA + N_CORES - 1) // N_CORES
    Rc += Rc & 1                               # even row count (pair loop)

    ga, gb = _build_tables(W, b)
    tha, thb = _thresholds()
    tb = _dist_bins(x_t)
    sb = _dist_bins(x_sc)

    pos = np.arange(P)
    pos_t = pos // 128
    pos_p = pos % 128

    cores = []
    meta = []
    for c in range(N_CORES):
        rows_c = A[c::N_CORES]
        nr = len(rows_c)
        rows = np.full(Rc, rows_c[-1] if nr else A[0], dtype=np.int64)
        rows[:nr] = rows_c

        lo = np.searchsorted(A, rows - 62, side="left")
        pb = np.clip(lo // 128, 0, JT - 2)

        # processed position -> packed index (rotation by pb tiles)
        ptrue = ((pb[:, None] + pos_t[None, :]) % JT) * 128 + pos_p[None, :]
        jtrue = Ap[ptrue]                          # [Rc, P] true col ids

        tbv = tb[rows[:, None], jtrue]             # [Rc, P]
        sbv = sb[rows[:, None], jtrue]
        vv = ptrue - 128 * pb[:, None]             # packed idx - window start
        bcb = np.empty((Rc, KB, P), np.float32)
        bcb[:, 0:29, :] = tbv[:, None, :]
        bcb[:, 29:58, :] = sbv[:, None, :]
        bcb[:, 58:60, :] = -vv[:, None, :]
        bcb[:, 60:62, :] = 1.0

        jwin = jtrue[:, 0:256]                     # window cols (natural order)
        u = (rows[:, None] - jwin).astype(np.float32)  # i - j
        bca = np.broadcast_to(u[:, None, :], (Rc, KA, 256))

        R2 = Rc // 2
        bcb_p = (bcb.reshape(R2, 2, KB, P).transpose(0, 2, 1, 3)
                 .reshape(R2 * KB, 2 * P))
        bca_p = (np.ascontiguousarray(bca).reshape(R2, 2, KA, 256)
                 .transpose(0, 2, 1, 3).reshape(R2 * KA, 512))
        cores.append({
            "ga": np.ascontiguousarray(ga),
            "gb": np.ascontiguousarray(gb),
            "tha": tha,
            "thb": thb,
            "bcb": np.ascontiguousarray(bcb_p).astype(BF16),
            "bca": np.ascontiguousarray(bca_p).astype(BF16),
        })
        meta.append((rows_c, pb[:nr] if nr else pb[:0]))
    return cores, meta, A, nA, Rc, JT


def kernel(mask, x_t, x_sc, W, b, gamma, beta):
    global LAST_PROFILE
    from concourse.bass_utils import run_bass_kernel_spmd

    mask = np.asarray(mask)
    out = np.zeros((N, N, C_OUT), np.float32)
    host = _host_data(mask, x_t, x_sc, W, b)
    if host is not None:
        cores, meta, A, nA, Rc, JT = host
        P = JT * 128
        nc = _build_program(Rc, JT)

        trace = bool(int(os.environ.get("KERNEL_TRACE", "0")))
        tdir = os.environ.get("KERNEL_TRACE_DIR") or None
        if tdir:
            os.makedirs(tdir, exist_ok=True)
        res = run_bass_kernel_spmd(nc, cores, list(range(N_CORES)), trace=trace,
                                   tmpdir=tdir)
        LAST_PROFILE = res

        for c in range(N_CORES):
            oc = res.results[c]["out"]             # [Rc, 128, JT*256] bf16
            rows_c, pbs = meta[c]
            for r, (i, pbr) in enumerate(zip(rows_c, pbs)):
                blk = oc[r].reshape(128, JT, C_OUT).transpose(1, 0, 2)
                if pbr:
                    blk = np.roll(blk, pbr, axis=0)
                out[i, A] = blk.reshape(P, C_OUT)[:nA].astype(np.float32)

    gamma = np.asarray(gamma, np.float32)
    beta = np.asarray(beta, np.float32)
    if not (np.all(gamma == 1.0) and np.all(beta == 0.0)):
        pm = (mask.astype(np.float32)[:, None] * mask.astype(np.float32)[None, :])
        out = out * gamma[None, None, :] + pm[:, :, None] * beta[None, None, :]
    return out
